# revision 8
# baseline (speedup 1.0000x reference)
"""CConv (continuous conv / GNN message passing) Trainium2 Bass kernel.

Math (per point n):
    pf[n,m,:]  = feat_in[neighbor_idx[n,m], :]                 # gather
    t[n,s,i]   = sum_m select_mat[n,m,s] * pf[n,m,i]           # stage 1
    out[n,o]   = sum_{s,i} t[n,s,i] * W[s,o,i]                 # stage 2

Strategy: data-parallel over points across 8 cores; per core, 49 groups of
128 points (32 blocks of 4 points). The neighbor gather is done host-side
(indirect DMA on this toolchain is limited to 128 rows/call) and shipped as
a contiguous bf16 stream. Stage 1 runs as one matmul per 4-point block
against a block-diagonal select operand with nb-major columns (nb*28+s) so
PSUM evictions into Tg[point*28+s] are fully contiguous; stage 2 reads Tg
with a stride-28 lhsT AP. The PE stream is software-pipelined by group
PAIRS: stage-2 of pair k-1 runs between stage-1 of pair k and k+1, with the
two groups' stage-2 accumulations interleaved across two PSUM banks, so PE
never waits on evictions and stays ramped. Select expansion: GpSimd takes
whole groups (one fat op, ~20/49), DVE (2x mode on bf16, after an ACT
uint8->bf16 convert) takes the rest. Evictions split ACT 5 : DVE 3.
Input DMAs ride the SP ring; output DMAs ride ACT's ring right after the
po->ot copy so no queue ever stalls on an unmet output dependency.
"""
import sys

sys.path.insert(0, '/opt/trn_rl_repo')

import numpy as np
import ml_dtypes

import concourse.bass as bass
import concourse.tile as tile
from concourse import bacc, mybir
from concourse.bass_utils import run_bass_kernel_spmd

BF16 = ml_dtypes.bfloat16

N = 50000
M = 32            # neighbors per point
S = 27            # spatial bins
SP = 28           # padded spatial (even, 4B-aligned bf16 runs)
I = 128           # in channels
O = 128           # out channels
NCORES = 8
NPAD = 50176      # 8 * 49 * 128
NPC = NPAD // NCORES        # 6272 points per core
G = NPC // 128              # 49 groups of 128 points
B = 32                      # 4-point blocks per group
SUB = 8                     # blocks accumulated per PSUM tile (2 banks)
BD = 4 * SP                 # block-diag columns per block (112)


def _is_gp_group(g):
    # ~20 of 49 groups get their expansion on GpSimd (one fat op each)
    return g % 5 in (1, 3)


def build_nc():
    nc = bacc.Bacc("TRN2", target_bir_lowering=False, debug=False)

    pfp = nc.dram_tensor("pfp", [G, 128, B * I], mybir.dt.bfloat16, kind="ExternalInput")
    selp = nc.dram_tensor("selp", [G, 128, B * SP], mybir.dt.uint8, kind="ExternalInput")
    wt = nc.dram_tensor("wt", [I, S * O], mybir.dt.bfloat16, kind="ExternalInput")
    maskc = nc.dram_tensor("maskc", [128, BD], mybir.dt.bfloat16, kind="ExternalInput")
    outp = nc.dram_tensor("outp", [NPC, O], mybir.dt.bfloat16, kind="ExternalOutput")

    with tile.TileContext(nc) as tc:
        with (
            tc.tile_pool(name="const", bufs=1) as const_pool,
            tc.tile_pool(name="work", bufs=6) as work,
            tc.tile_pool(name="tgp", bufs=4) as tgp,
            tc.tile_pool(name="psum1", bufs=3, space="PSUM") as psum1,
            tc.tile_pool(name="psum2", bufs=2, space="PSUM") as psum2,
        ):
            wt_t = const_pool.tile([128, S * O], mybir.dt.bfloat16)
            nc.scalar.dma_start(out=wt_t[:], in_=wt[:])
            mask_t = const_pool.tile([128, BD], mybir.dt.bfloat16)
            nc.scalar.dma_start(out=mask_t[:], in_=maskc[:])

            def expand(eng, src_t, rhs_t):
                # rhs_t[q, b, nb*28+s] = src_t[q, b*28+s] * mask[q, nb*28+s]
                out_ap = bass.AP(tensor=rhs_t.tensor, offset=rhs_t[:].offset,
                                 ap=[rhs_t[:].ap[0], [BD, B], [SP, 4], [1, SP]])
                in0_ap = bass.AP(tensor=src_t.tensor, offset=src_t[:].offset,
                                 ap=[src_t[:].ap[0], [SP, B], [0, 4], [1, SP]])
                in1_ap = bass.AP(tensor=mask_t.tensor, offset=mask_t[:].offset,
                                 ap=[mask_t[:].ap[0], [0, B], [SP, 4], [1, SP]])
                eng.tensor_tensor(out=out_ap, in0=in0_ap, in1=in1_ap,
                                  op=mybir.AluOpType.mult)

            # eviction engine pattern over the 8 c-tiles of a pair: 5 ACT, 3 DVE
            EV_PAT = ['A', 'A', 'D', 'A', 'D', 'A', 'A', 'D']

            def stage1(g):
                """Emit loads, expansion, stage-1 matmuls and evictions for
                group g. Returns the Tg tile holding t[point*28+s]."""
                sel_t = work.tile([128, B * SP], mybir.dt.uint8)
                nc.sync.dma_start(out=sel_t[:], in_=selp[g])
                pf_t = work.tile([128, B, I], mybir.dt.bfloat16)
                nc.sync.dma_start(out=pf_t[:], in_=pfp[g])

                rhs_t = work.tile([128, B, BD], mybir.dt.bfloat16)
                if _is_gp_group(g):
                    expand(nc.gpsimd, sel_t, rhs_t)
                else:
                    selb_t = work.tile([128, B * SP], mybir.dt.bfloat16)
                    nc.scalar.copy(out=selb_t[:], in_=sel_t[:])
                    expand(nc.vector, selb_t, rhs_t)

                Tg = tgp.tile([128, 128 * SP], mybir.dt.bfloat16)
                for c in range(B // SUB):
                    pt = psum1.tile([128, SUB, 128], mybir.dt.float32, space="PSUM")
                    for sub in range(SUB):
                        b = c * SUB + sub
                        nc.tensor.matmul(
                            out=pt[:, sub, 0:BD],
                            lhsT=pf_t[:, b, :],
                            rhs=rhs_t[:, b, :],
                            start=True, stop=True,
                        )
                    # contiguous eviction: dst col = point*28+s, point=(c*8+sub)*4+nb
                    src_ap = bass.AP(tensor=pt.tensor, offset=pt[:].offset,
                                     ap=[pt[:].ap[0], [128, SUB], [1, BD]])
                    dst_ap = bass.AP(tensor=Tg.tensor,
                                     offset=Tg[:].offset + c * SUB * BD,
                                     ap=[Tg[:].ap[0], [BD, SUB], [1, BD]])
                    ev = EV_PAT[(g % 2) * 4 + c]
                    if ev == 'A':
                        nc.scalar.copy(out=dst_ap, in_=src_ap)
                    else:
                        nc.vector.tensor_copy(out=dst_ap, in_=src_ap)
                return Tg

            def stage2_pair(pair):
                """Interleaved stage-2 for a pair of (g, Tg) entries; returns
                list of (g, po)."""
                pos = [(g, psum2.tile([128, O], mybir.dt.float32, space="PSUM",
                                      name="po"))
                       for g, _ in pair]
                for s in range(S):
                    for (g, Tg), (_, po) in zip(pair, pos):
                        lhs_ap = bass.AP(tensor=Tg.tensor, offset=Tg[:].offset + s,
                                         ap=[Tg[:].ap[0], [SP, 128]])
                        nc.tensor.matmul(
                            out=po[:],
                            lhsT=lhs_ap,
                            rhs=wt_t[:, s * O:(s + 1) * O],
                            start=(s == 0), stop=(s == S - 1),
                            skip_group_check=True,
                        )
                return pos

            def flush(pos):
                for g, po in pos:
                    ot = work.tile([128, O], mybir.dt.bfloat16)
                    nc.scalar.copy(out=ot[:], in_=po[:])
                    nc.scalar.dma_start(out=outp[g * 128:(g + 1) * 128, :], in_=ot[:])

            pairs = [tuple(range(k, min(k + 2, G))) for k in range(0, G, 2)]
            prev = None
            for pi, pr in enumerate(pairs):
                cur = [(g, stage1(g)) for g in pr]
                if prev is not None:
                    flush(stage2_pair(prev))
                prev = cur
            flush(stage2_pair(prev))

    nc.compile()
    return nc


_NC = None


def get_nc():
    global _NC
    if _NC is None:
        _NC = build_nc()
    return _NC


def make_in_maps(feat_in, select_mat, weight, neighbor_idx):
    featb_np = np.asarray(feat_in, dtype=np.float32).astype(BF16)

    sel = np.asarray(select_mat, dtype=np.float32)
    sel_pad = np.zeros((NPAD, M, SP), dtype=np.float32)
    sel_pad[:N, :, :S] = sel

    nidx = np.asarray(neighbor_idx).astype(np.int64)
    idx_pad = np.zeros((NPAD, M), dtype=np.int64)
    idx_pad[:N] = nidx

    w = np.asarray(weight, dtype=np.float32)
    wt_np = np.ascontiguousarray(
        w.reshape(S, O, I).transpose(2, 0, 1).reshape(I, S * O)).astype(BF16)

    q = np.arange(128)[:, None]
    c = np.arange(BD)[None, :]
    mask_np = ((q // 32 == c // SP) / 256.0).astype(BF16)

    in_maps = []
    for core in range(NCORES):
        lo = core * NPC
        selc = sel_pad[lo:lo + NPC]
        idxc = idx_pad[lo:lo + NPC]
        # selp[g, nb*32+m, b*SP+s] = round(sel[g*128 + b*4 + nb, m, s] * 256)
        # (uint8 fixed-point; the 1/256 dequant is folded into the mask)
        selp_np = np.clip(np.rint(np.ascontiguousarray(
            selc.reshape(G, B, 4, M, SP).transpose(0, 2, 3, 1, 4)
        ).reshape(G, 128, B * SP) * 256.0), 0, 255).astype(np.uint8)
        # idxp[g, nb*32+m, b] = neighbor_idx[g*128 + b*4 + nb, m]
        idxp = np.ascontiguousarray(
            idxc.reshape(G, B, 4, M).transpose(0, 2, 3, 1))  # [G, 128, B]
        # host gather: pfp[g, q, b, :] = featb[idxp[g, q, b]]
        pfp_np = featb_np[idxp].reshape(G, 128, B * I)
        in_maps.append({
            "pfp": pfp_np,
            "selp": selp_np,
            "wt": wt_np,
            "maskc": mask_np,
        })
    return in_maps


def run(feat_in, select_mat, weight, neighbor_idx, trace=False):
    nc = get_nc()
    in_maps = make_in_maps(feat_in, select_mat, weight, neighbor_idx)
    res = run_bass_kernel_spmd(nc, in_maps, core_ids=list(range(NCORES)), trace=trace)
    outs = [res.results[c]["outp"] for c in range(NCORES)]
    full = np.concatenate(outs, axis=0)[:N].astype(np.float32)   # [N, O]
    return full[:, :, None], res


def kernel(feat_in, select_mat, weight, neighbor_idx):
    out, _ = run(feat_in, select_mat, weight, neighbor_idx, trace=False)
    return out


# revision 9
# speedup vs baseline: 1.0390x; 1.0390x over previous
"""CConv (continuous conv / GNN message passing) Trainium2 Bass kernel.

Math (per point n):
    pf[n,m,:]  = feat_in[neighbor_idx[n,m], :]                 # gather
    t[n,s,i]   = sum_m select_mat[n,m,s] * pf[n,m,i]           # stage 1
    out[n,o]   = sum_{s,i} t[n,s,i] * W[s,o,i]                 # stage 2

Strategy: data-parallel over points across 8 cores; per core, 49 groups of
128 points (32 blocks of 4 points). The neighbor gather is done host-side
(indirect DMA on this toolchain is limited to 128 rows/call) and shipped as
a contiguous bf16 stream. Stage 1 runs as one matmul per 4-point block
against a block-diagonal select operand with nb-major columns (nb*27+s) so
PSUM evictions into Tg[point*27+s] are fully contiguous; stage 2 reads Tg
with a stride-27 lhsT AP. The schedule is software-pipelined by group
PAIRS (phases): phase k emits loads for phase k+1, stage-1 of phase k,
select-expansions for phase k+1 (after this phase's evictions in DVE
program order, so PSUM frees promptly), stage-2 of pair k-1 (interleaved
across two PSUM accumulators), and output flush of pair k-1. Expansions:
GpSimd takes whole groups (one fat op, 20/49) from uint8 sel; DVE (2x
mode) takes the rest from a pre-scaled bf16 sel, sharing one 1/256 mask.
Evictions split ACT 5 : DVE 3. Input DMAs ride the SP ring; output DMAs
ride ACT's ring right after the po->ot copy.
"""
import sys

sys.path.insert(0, '/opt/trn_rl_repo')

import numpy as np
import ml_dtypes

import concourse.bass as bass
import concourse.tile as tile
from concourse import bacc, mybir
from concourse.bass_utils import run_bass_kernel_spmd

BF16 = ml_dtypes.bfloat16

N = 50000
M = 32            # neighbors per point
S = 27            # spatial bins
SP = 28           # padded spatial stride in shipped sel (DMA alignment)
I = 128           # in channels
O = 128           # out channels
NCORES = 8
NPAD = 50176      # 8 * 49 * 128
NPC = NPAD // NCORES        # 6272 points per core
G = NPC // 128              # 49 groups of 128 points
B = 32                      # 4-point blocks per group
SUB = 8                     # blocks accumulated per PSUM tile (2 banks)
BD = 4 * S                  # block-diag columns per block (108)


def _is_gp_group(g):
    # 20 of 49 groups get their expansion on GpSimd (one fat op each)
    return g % 5 in (1, 3)


def build_nc():
    nc = bacc.Bacc("TRN2", target_bir_lowering=False, debug=False)

    pfp = nc.dram_tensor("pfp", [G, 128, B * I], mybir.dt.bfloat16, kind="ExternalInput")
    selp8 = nc.dram_tensor("selp8", [G, 128, B * SP], mybir.dt.uint8, kind="ExternalInput")
    selp16 = nc.dram_tensor("selp16", [G, 128, B * SP], mybir.dt.bfloat16, kind="ExternalInput")
    wt = nc.dram_tensor("wt", [I, S * O], mybir.dt.bfloat16, kind="ExternalInput")
    maskc = nc.dram_tensor("maskc", [128, BD], mybir.dt.bfloat16, kind="ExternalInput")
    outp = nc.dram_tensor("outp", [NPC, O], mybir.dt.bfloat16, kind="ExternalOutput")

    with tile.TileContext(nc) as tc:
        with (
            tc.tile_pool(name="const", bufs=1) as const_pool,
            tc.tile_pool(name="work", bufs=6) as work,
            tc.tile_pool(name="tgp", bufs=5) as tgp,
            tc.tile_pool(name="psum1", bufs=3, space="PSUM") as psum1,
            tc.tile_pool(name="psum2", bufs=2, space="PSUM") as psum2,
        ):
            wt_t = const_pool.tile([128, S * O], mybir.dt.bfloat16)
            nc.scalar.dma_start(out=wt_t[:], in_=wt[:])
            mask_t = const_pool.tile([128, BD], mybir.dt.bfloat16)
            nc.scalar.dma_start(out=mask_t[:], in_=maskc[:])

            # eviction engine pattern over the 8 c-tiles of a pair: 5 ACT, 3 DVE
            EV_PAT = ['A', 'A', 'D', 'A', 'D', 'A', 'A', 'D']

            def loads(g):
                if _is_gp_group(g):
                    sel_t = work.tile([128, B * SP], mybir.dt.uint8, name="sel8")
                    nc.sync.dma_start(out=sel_t[:], in_=selp8[g])
                else:
                    sel_t = work.tile([128, B * SP], mybir.dt.bfloat16, name="sel16")
                    nc.sync.dma_start(out=sel_t[:], in_=selp16[g])
                pf_t = work.tile([128, B, I], mybir.dt.bfloat16, name="pf")
                nc.sync.dma_start(out=pf_t[:], in_=pfp[g])
                return sel_t, pf_t

            def expand(g, sel_t):
                # rhs_t[q, b, nb*27+s] = sel_t[q, b*28+s] * mask[q, nb*27+s]
                rhs_t = work.tile([128, B, BD], mybir.dt.bfloat16, name="rhs")
                out_ap = bass.AP(tensor=rhs_t.tensor, offset=rhs_t[:].offset,
                                 ap=[rhs_t[:].ap[0], [BD, B], [S, 4], [1, S]])
                in0_ap = bass.AP(tensor=sel_t.tensor, offset=sel_t[:].offset,
                                 ap=[sel_t[:].ap[0], [SP, B], [0, 4], [1, S]])
                in1_ap = bass.AP(tensor=mask_t.tensor, offset=mask_t[:].offset,
                                 ap=[mask_t[:].ap[0], [0, B], [S, 4], [1, S]])
                eng = nc.gpsimd if _is_gp_group(g) else nc.vector
                eng.tensor_tensor(out=out_ap, in0=in0_ap, in1=in1_ap,
                                  op=mybir.AluOpType.mult)
                return rhs_t

            def stage1(g, pf_t, rhs_t):
                """Stage-1 matmuls + contiguous evictions -> Tg[point*27+s]."""
                Tg = tgp.tile([128, 128 * S], mybir.dt.bfloat16, name="Tg")
                for c in range(B // SUB):
                    pt = psum1.tile([128, SUB, 128], mybir.dt.float32,
                                    space="PSUM", name="pt")
                    for sub in range(SUB):
                        b = c * SUB + sub
                        nc.tensor.matmul(
                            out=pt[:, sub, 0:BD],
                            lhsT=pf_t[:, b, :],
                            rhs=rhs_t[:, b, :],
                            start=True, stop=True,
                        )
                    src_ap = bass.AP(tensor=pt.tensor, offset=pt[:].offset,
                                     ap=[pt[:].ap[0], [128, SUB], [1, BD]])
                    dst_ap = bass.AP(tensor=Tg.tensor,
                                     offset=Tg[:].offset + c * SUB * BD,
                                     ap=[Tg[:].ap[0], [BD, SUB], [1, BD]])
                    if EV_PAT[(g % 2) * 4 + c] == 'A':
                        nc.scalar.copy(out=dst_ap, in_=src_ap)
                    else:
                        nc.vector.tensor_copy(out=dst_ap, in_=src_ap)
                return Tg

            def stage2_pair(pair):
                pos = [(g, psum2.tile([128, O], mybir.dt.float32, space="PSUM",
                                      name="po"))
                       for g, _ in pair]
                for s in range(S):
                    for (g, Tg), (_, po) in zip(pair, pos):
                        lhs_ap = bass.AP(tensor=Tg.tensor, offset=Tg[:].offset + s,
                                         ap=[Tg[:].ap[0], [S, 128]])
                        nc.tensor.matmul(
                            out=po[:],
                            lhsT=lhs_ap,
                            rhs=wt_t[:, s * O:(s + 1) * O],
                            start=(s == 0), stop=(s == S - 1),
                            skip_group_check=True,
                        )
                return pos

            def flush(pos):
                for g, po in pos:
                    ot = work.tile([128, O], mybir.dt.bfloat16, name="ot")
                    nc.scalar.copy(out=ot[:], in_=po[:])
                    nc.scalar.dma_start(out=outp[g * 128:(g + 1) * 128, :], in_=ot[:])

            phases = [tuple(range(k, min(k + 2, G))) for k in range(0, G, 2)]
            # warmup: loads + expansions for phase 0
            ld = {}
            rhs = {}
            for g in phases[0]:
                ld[g] = loads(g)
            for g in phases[0]:
                rhs[g] = expand(g, ld[g][0])

            prev = None
            for k, pr in enumerate(phases):
                if k + 1 < len(phases):
                    for g in phases[k + 1]:
                        ld[g] = loads(g)
                cur = [(g, stage1(g, ld[g][1], rhs[g])) for g in pr]
                for g in pr:
                    del ld[g], rhs[g]
                if k + 1 < len(phases):
                    for g in phases[k + 1]:
                        rhs[g] = expand(g, ld[g][0])
                if prev is not None:
                    flush(stage2_pair(prev))
                prev = cur
            flush(stage2_pair(prev))

    nc.compile()
    return nc


_NC = None


def get_nc():
    global _NC
    if _NC is None:
        _NC = build_nc()
    return _NC


def make_in_maps(feat_in, select_mat, weight, neighbor_idx):
    featb_np = np.asarray(feat_in, dtype=np.float32).astype(BF16)

    sel = np.asarray(select_mat, dtype=np.float32)
    sel_pad = np.zeros((NPAD, M, SP), dtype=np.float32)
    sel_pad[:N, :, :S] = sel

    nidx = np.asarray(neighbor_idx).astype(np.int64)
    idx_pad = np.zeros((NPAD, M), dtype=np.int64)
    idx_pad[:N] = nidx

    w = np.asarray(weight, dtype=np.float32)
    wt_np = np.ascontiguousarray(
        w.reshape(S, O, I).transpose(2, 0, 1).reshape(I, S * O)).astype(BF16)

    q = np.arange(128)[:, None]
    c = np.arange(BD)[None, :]
    mask_np = ((q // 32 == c // S) / 256.0).astype(BF16)

    in_maps = []
    for core in range(NCORES):
        lo = core * NPC
        selc = sel_pad[lo:lo + NPC]
        idxc = idx_pad[lo:lo + NPC]
        # selq[g, nb*32+m, b*SP+s] = sel[g*128 + b*4 + nb, m, s] * 256
        selq = np.ascontiguousarray(
            selc.reshape(G, B, 4, M, SP).transpose(0, 2, 3, 1, 4)
        ).reshape(G, 128, B * SP) * 256.0
        # uint8 fixed-point for GpSimd groups; the 1/256 dequant lives in the mask
        selp8_np = np.clip(np.rint(selq), 0, 255).astype(np.uint8)
        # bf16 pre-scaled (x256, exact power-of-2) for DVE groups, same mask
        selp16_np = selq.astype(BF16)
        # idxp[g, nb*32+m, b] = neighbor_idx[g*128 + b*4 + nb, m]
        idxp = np.ascontiguousarray(
            idxc.reshape(G, B, 4, M).transpose(0, 2, 3, 1))  # [G, 128, B]
        # host gather: pfp[g, q, b, :] = featb[idxp[g, q, b]]
        pfp_np = featb_np[idxp].reshape(G, 128, B * I)
        in_maps.append({
            "pfp": pfp_np,
            "selp8": selp8_np,
            "selp16": selp16_np,
            "wt": wt_np,
            "maskc": mask_np,
        })
    return in_maps


def run(feat_in, select_mat, weight, neighbor_idx, trace=False):
    nc = get_nc()
    in_maps = make_in_maps(feat_in, select_mat, weight, neighbor_idx)
    res = run_bass_kernel_spmd(nc, in_maps, core_ids=list(range(NCORES)), trace=trace)
    outs = [res.results[c]["outp"] for c in range(NCORES)]
    full = np.concatenate(outs, axis=0)[:N].astype(np.float32)   # [N, O]
    return full[:, :, None], res


def kernel(feat_in, select_mat, weight, neighbor_idx):
    out, _ = run(feat_in, select_mat, weight, neighbor_idx, trace=False)
    return out


# revision 13
# speedup vs baseline: 1.1672x; 1.1234x over previous
"""CConv (continuous conv / GNN message passing) Trainium2 Bass kernel.

Math (per point n):
    pf[n,m,:]  = feat_in[neighbor_idx[n,m], :]                 # gather
    t[n,s,i]   = sum_m select_mat[n,m,s] * pf[n,m,i]           # stage 1
    out[n,o]   = sum_{s,i} t[n,s,i] * W[s,o,i]                 # stage 2

Strategy: data-parallel over points across 8 cores; per core, 49 groups of
128 points (32 blocks of 4 points). The neighbor gather is done host-side
(indirect DMA on this toolchain is limited to 128 rows/call) and shipped as
a contiguous bf16 stream. Stage 1 runs as one matmul per 4-point block
against a block-diagonal select operand with nb-major columns (nb*27+s) so
PSUM evictions into Tg[point*27+s] are fully contiguous; stage 2 reads Tg
with a stride-27 lhsT AP. The schedule is software-pipelined by group
PAIRS (phases): phase k emits loads for phase k+1, stage-1 of phase k,
select-expansions for phase k+1 (after this phase's evictions in DVE
program order, so PSUM frees promptly), stage-2 of pair k-1 (interleaved
across two PSUM accumulators), and output flush of pair k-1. Expansions:
GpSimd takes whole groups (one fat op, 20/49) from uint8 sel; DVE (2x
mode) takes the rest from a pre-scaled bf16 sel, sharing one 1/256 mask.
Evictions split ACT 5 : DVE 3. Input DMAs ride the SP ring; output DMAs
ride ACT's ring right after the po->ot copy.
"""
import sys

sys.path.insert(0, '/opt/trn_rl_repo')

import numpy as np
import ml_dtypes

import concourse.bass as bass
import concourse.tile as tile
from concourse import bacc, mybir
from concourse.bass_utils import run_bass_kernel_spmd

BF16 = ml_dtypes.bfloat16

N = 50000
M = 32            # neighbors per point
S = 27            # spatial bins
SP = 28           # padded spatial stride in shipped sel (DMA alignment)
I = 128           # in channels
O = 128           # out channels
NCORES = 8
NPAD = 50176      # 8 * 49 * 128
NPC = NPAD // NCORES        # 6272 points per core
G = NPC // 128              # 49 groups of 128 points
B = 32                      # 4-point blocks per group
SUB = 8                     # blocks accumulated per PSUM tile (2 banks)
BD = 4 * S                  # block-diag columns per block (108)


def _is_gp_group(g):
    # odd groups (24 of 49) get their expansion on GpSimd (one fat op each)
    return g % 2 == 1


def build_nc():
    nc = bacc.Bacc("TRN2", target_bir_lowering=False, debug=False)

    pfp = nc.dram_tensor("pfp", [G, 128, B * I], mybir.dt.bfloat16, kind="ExternalInput")
    selp8 = nc.dram_tensor("selp8", [G, 128, B * SP], mybir.dt.uint8, kind="ExternalInput")
    selp16 = nc.dram_tensor("selp16", [G, 128, B * SP], mybir.dt.bfloat16, kind="ExternalInput")
    wt = nc.dram_tensor("wt", [I, S * O], mybir.dt.bfloat16, kind="ExternalInput")
    maskc = nc.dram_tensor("maskc", [128, BD], mybir.dt.bfloat16, kind="ExternalInput")
    outp = nc.dram_tensor("outp", [NPC, O], mybir.dt.bfloat16, kind="ExternalOutput")

    with tile.TileContext(nc) as tc:
        with (
            tc.tile_pool(name="const", bufs=1) as const_pool,
            tc.tile_pool(name="work", bufs=7) as work,
            tc.tile_pool(name="tgp", bufs=5) as tgp,
            tc.tile_pool(name="psum1", bufs=3, space="PSUM") as psum1,
            tc.tile_pool(name="psum2", bufs=2, space="PSUM") as psum2,
        ):
            wt_t = const_pool.tile([128, S * O], mybir.dt.bfloat16)
            nc.scalar.dma_start(out=wt_t[:], in_=wt[:])
            mask_t = const_pool.tile([128, BD], mybir.dt.bfloat16)
            nc.scalar.dma_start(out=mask_t[:], in_=maskc[:])

            # eviction engine pattern over the 8 c-tiles of a pair: 6 ACT, 2 DVE
            EV_PAT = ['A', 'A', 'D', 'A', 'A', 'A', 'A', 'D']

            def loads(g):
                if _is_gp_group(g):
                    sel_t = work.tile([128, B * SP], mybir.dt.uint8, name="sel8")
                    nc.sync.dma_start(out=sel_t[:], in_=selp8[g])
                else:
                    sel_t = work.tile([128, B * SP], mybir.dt.bfloat16, name="sel16")
                    nc.sync.dma_start(out=sel_t[:], in_=selp16[g])
                pf_t = work.tile([128, B, I], mybir.dt.bfloat16, name="pf")
                nc.sync.dma_start(out=pf_t[:], in_=pfp[g])
                return sel_t, pf_t

            def expand(g, sel_t):
                # rhs_t[q, b, nb*27+s] = sel_t[q, b*28+s] * mask[q, nb*27+s]
                rhs_t = work.tile([128, B, BD], mybir.dt.bfloat16, name="rhs")
                out_ap = bass.AP(tensor=rhs_t.tensor, offset=rhs_t[:].offset,
                                 ap=[rhs_t[:].ap[0], [BD, B], [S, 4], [1, S]])
                in0_ap = bass.AP(tensor=sel_t.tensor, offset=sel_t[:].offset,
                                 ap=[sel_t[:].ap[0], [SP, B], [0, 4], [1, S]])
                in1_ap = bass.AP(tensor=mask_t.tensor, offset=mask_t[:].offset,
                                 ap=[mask_t[:].ap[0], [0, B], [S, 4], [1, S]])
                eng = nc.gpsimd if _is_gp_group(g) else nc.vector
                eng.tensor_tensor(out=out_ap, in0=in0_ap, in1=in1_ap,
                                  op=mybir.AluOpType.mult)
                return rhs_t

            def stage1(g, pf_t, rhs_t):
                """Stage-1 matmuls + contiguous evictions -> Tg[point*27+s]."""
                Tg = tgp.tile([128, 128 * S], mybir.dt.bfloat16, name="Tg")
                for c in range(B // SUB):
                    pt = psum1.tile([128, SUB, 128], mybir.dt.float32,
                                    space="PSUM", name="pt")
                    for sub in range(SUB):
                        b = c * SUB + sub
                        nc.tensor.matmul(
                            out=pt[:, sub, 0:BD],
                            lhsT=pf_t[:, b, :],
                            rhs=rhs_t[:, b, :],
                            start=True, stop=True,
                        )
                    src_ap = bass.AP(tensor=pt.tensor, offset=pt[:].offset,
                                     ap=[pt[:].ap[0], [128, SUB], [1, BD]])
                    dst_ap = bass.AP(tensor=Tg.tensor,
                                     offset=Tg[:].offset + c * SUB * BD,
                                     ap=[Tg[:].ap[0], [BD, SUB], [1, BD]])
                    if EV_PAT[(g % 2) * 4 + c] == 'A':
                        nc.scalar.copy(out=dst_ap, in_=src_ap)
                    else:
                        nc.vector.tensor_copy(out=dst_ap, in_=src_ap)
                return Tg

            def stage2_pair(pair):
                pos = [(g, psum2.tile([128, O], mybir.dt.float32, space="PSUM",
                                      name="po"))
                       for g, _ in pair]
                for s in range(S):
                    for (g, Tg), (_, po) in zip(pair, pos):
                        lhs_ap = bass.AP(tensor=Tg.tensor, offset=Tg[:].offset + s,
                                         ap=[Tg[:].ap[0], [S, 128]])
                        nc.tensor.matmul(
                            out=po[:],
                            lhsT=lhs_ap,
                            rhs=wt_t[:, s * O:(s + 1) * O],
                            start=(s == 0), stop=(s == S - 1),
                            skip_group_check=True,
                        )
                return pos

            def flush(pos):
                for g, po in pos:
                    ot = work.tile([128, O], mybir.dt.bfloat16, name="ot")
                    nc.scalar.copy(out=ot[:], in_=po[:])
                    nc.scalar.dma_start(out=outp[g * 128:(g + 1) * 128, :], in_=ot[:])

            phases = [tuple(range(k, min(k + 2, G))) for k in range(0, G, 2)]
            # warmup: loads for phases 0-1, expansions for phase 0
            ld = {}
            rhs = {}
            for ph in phases[:2]:
                for g in ph:
                    ld[g] = loads(g)
            for g in phases[0]:
                rhs[g] = expand(g, ld[g][0])

            prev = None
            for k, pr in enumerate(phases):
                if k + 2 < len(phases):
                    for g in phases[k + 2]:
                        ld[g] = loads(g)
                cur = [(g, stage1(g, ld[g][1], rhs[g])) for g in pr]
                for g in pr:
                    del ld[g], rhs[g]
                if k + 1 < len(phases):
                    for g in phases[k + 1]:
                        rhs[g] = expand(g, ld[g][0])
                if prev is not None:
                    flush(stage2_pair(prev))
                prev = cur
            flush(stage2_pair(prev))

    nc.compile()
    return nc


_NC = None


def get_nc():
    global _NC
    if _NC is None:
        _NC = build_nc()
    return _NC


def make_in_maps(feat_in, select_mat, weight, neighbor_idx):
    featb_np = np.asarray(feat_in, dtype=np.float32).astype(BF16)

    sel = np.asarray(select_mat, dtype=np.float32)
    sel_pad = np.zeros((NPAD, M, SP), dtype=np.float32)
    sel_pad[:N, :, :S] = sel

    nidx = np.asarray(neighbor_idx).astype(np.int64)
    idx_pad = np.zeros((NPAD, M), dtype=np.int64)
    idx_pad[:N] = nidx

    w = np.asarray(weight, dtype=np.float32)
    wt_np = np.ascontiguousarray(
        w.reshape(S, O, I).transpose(2, 0, 1).reshape(I, S * O)).astype(BF16)

    q = np.arange(128)[:, None]
    c = np.arange(BD)[None, :]
    mask_np = ((q // 32 == c // S) / 256.0).astype(BF16)

    in_maps = []
    for core in range(NCORES):
        lo = core * NPC
        selc = sel_pad[lo:lo + NPC]
        idxc = idx_pad[lo:lo + NPC]
        # selq[g, nb*32+m, b*SP+s] = sel[g*128 + b*4 + nb, m, s] * 256
        selq = np.ascontiguousarray(
            selc.reshape(G, B, 4, M, SP).transpose(0, 2, 3, 1, 4)
        ).reshape(G, 128, B * SP) * 256.0
        # uint8 fixed-point for GpSimd groups; the 1/256 dequant lives in the mask
        selp8_np = np.clip(np.rint(selq), 0, 255).astype(np.uint8)
        # bf16 pre-scaled (x256, exact power-of-2) for DVE groups, same mask
        selp16_np = selq.astype(BF16)
        # idxp[g, nb*32+m, b] = neighbor_idx[g*128 + b*4 + nb, m]
        idxp = np.ascontiguousarray(
            idxc.reshape(G, B, 4, M).transpose(0, 2, 3, 1))  # [G, 128, B]
        # host gather: pfp[g, q, b, :] = featb[idxp[g, q, b]]
        pfp_np = featb_np[idxp].reshape(G, 128, B * I)
        in_maps.append({
            "pfp": pfp_np,
            "selp8": selp8_np,
            "selp16": selp16_np,
            "wt": wt_np,
            "maskc": mask_np,
        })
    return in_maps


def run(feat_in, select_mat, weight, neighbor_idx, trace=False):
    nc = get_nc()
    in_maps = make_in_maps(feat_in, select_mat, weight, neighbor_idx)
    res = run_bass_kernel_spmd(nc, in_maps, core_ids=list(range(NCORES)), trace=trace)
    outs = [res.results[c]["outp"] for c in range(NCORES)]
    full = np.concatenate(outs, axis=0)[:N].astype(np.float32)   # [N, O]
    return full[:, :, None], res


def kernel(feat_in, select_mat, weight, neighbor_idx):
    out, _ = run(feat_in, select_mat, weight, neighbor_idx, trace=False)
    return out
